# revision 24
# baseline (speedup 1.0000x reference)
"""Multi-head attention (RoPE, 16 heads, D=1024, B=2, N=2048) on 8 trn2 cores.

Sharding: core c handles batch b = c//4 and heads [4*(c%4), 4*(c%4)+4).
Each core computes its 4 heads' attention plus its partial out-projection
(columns of Wo for its heads); host sums the 4 partials per batch.

Layout strategy (per core):
  - Q^T, K^T computed directly in (head_dim, tokens) layout; head_dim rows are
    permuted (evens then odds) via host-permuted Wq/Wk rows so RoPE becomes a
    rotate-half over partitions 0:32/32:64 of each 64-row head block.
  - scores^T tiles (k_tokens x q) via row-packed K=64 matmuls (head pairs at
    partitions 0:64 / 64:128 of shared Q^T/K^T tiles).
  - exp on ScalarE straight out of PSUM with fused 1/sqrt(hd) scale.
  - numerator matmul uses V augmented with a ones column (M=65): row 64 of the
    PSUM accumulator is the softmax denominator.
  - normalize: DVE reciprocal of row 64, gpsimd partition-broadcast, DVE mult.
  - out-projection: lhsT = normalized attention output (already transposed),
    rhs = Wo columns for this core's heads, accumulated over 2 head-pair chunks.
"""

import numpy as np

import concourse.bass as bass
import concourse.mybir as mybir
import concourse.tile as tile
from concourse import bacc
from concourse.bass_utils import run_bass_kernel_spmd

F32 = mybir.dt.float32
F32R = mybir.dt.float32r
AF = mybir.ActivationFunctionType
OP = mybir.AluOpType

B, N, D = 2, 2048, 1024
H, HD = 16, 64
HPC = 4  # heads per core
N_CORES = 8
SCALE = HD ** -0.5

N_TOKTILES = N // 128      # 16
N_KTILES = N // 128        # 16
N_QTILES = N // 512        # 4
N_DTILES = D // 128        # 8
KG = 2                     # ktiles per exp group
NEG = -1.0e30


def _build_program(mask_all_ones: bool):
    nc = bacc.Bacc("TRN2", target_bir_lowering=False, debug=False)

    xT = nc.dram_tensor("xT", [D, N], F32R, kind="ExternalInput")
    wqT = nc.dram_tensor("wqT", [D, HPC * HD], F32R, kind="ExternalInput")
    wkT = nc.dram_tensor("wkT", [D, HPC * HD], F32R, kind="ExternalInput")
    wvT = nc.dram_tensor("wvT", [D, HPC * HD], F32R, kind="ExternalInput")
    woT = nc.dram_tensor("woT", [HPC * HD, D], F32R, kind="ExternalInput")
    cosT = nc.dram_tensor("cosT", [128, N], F32, kind="ExternalInput")
    sinT = nc.dram_tensor("sinT", [128, N], F32, kind="ExternalInput")
    mmul = nc.dram_tensor("mmul", [128, N_KTILES], F32, kind="ExternalInput")
    y = nc.dram_tensor("y", [N, D], F32, kind="ExternalOutput")

    NQ = 512  # token quarter

    with tile.TileContext(nc) as tc:
        with (
            tc.tile_pool(name="qk", bufs=2 * HPC) as qk_pool,
            tc.tile_pool(name="vaug", bufs=N_TOKTILES) as v_pool,
            tc.tile_pool(name="outT", bufs=2) as outT_pool,
            tc.tile_pool(name="wo", bufs=2) as wo_pool,
            tc.tile_pool(name="mm", bufs=1) as mm_pool,
            tc.tile_pool(name="tab", bufs=2) as tab_pool,
            tc.tile_pool(name="raw", bufs=3) as raw_pool,
            tc.tile_pool(name="rot", bufs=3) as rot_pool,
            tc.tile_pool(name="xt", bufs=N_DTILES) as xt_pool,
            tc.tile_pool(name="w", bufs=N_DTILES) as w_pool,
            tc.tile_pool(name="on", bufs=1) as on_pool,
            tc.tile_pool(name="exp", bufs=3) as exp_pool,
            tc.tile_pool(name="div", bufs=4) as div_pool,
            tc.tile_pool(name="yout", bufs=2) as y_pool,
            tc.tile_pool(name="psumS", bufs=2, space="PSUM") as psumS,
            tc.tile_pool(name="psumN", bufs=2, space="PSUM") as psumN,
            tc.tile_pool(name="psumW", bufs=1, space="PSUM") as psumW,
        ):
            # QTp[h], KTp[h]: (128, N) f32r; rows 0:64 = head h, 64:128 = zeros
            # (zero-padded so every matmul has K=128 and counts as HAM-busy)
            QTp = [qk_pool.tile([128, N], F32R, tag="qk", name=f"QTp{_}") for _ in range(HPC)]
            KTp = [qk_pool.tile([128, N], F32R, tag="qk", name=f"KTp{_}") for _ in range(HPC)]
            vaug = [
                v_pool.tile([128, HPC * (HD + 1)], F32R, tag="vaug", name=f"vaug{_}")
                for _ in range(N_TOKTILES)
            ]
            outT = [outT_pool.tile([128, N], F32R, tag="outT", name=f"outT{_}") for _ in range(2)]
            woT_sb = [wo_pool.tile([128, D], F32R, tag="wo", name=f"woTsb{_}") for _ in range(2)]
            mmul_sb = mm_pool.tile([128, N_KTILES], F32)
            cos_sb = tab_pool.tile([128, N], F32, tag="tab")
            sin_sb = tab_pool.tile([128, N], F32, tag="tab")
            ones_sc = on_pool.tile([128, HPC], F32, tag="on1", name="ones_sc")
            zsrc = on_pool.tile([128, 512], F32, tag="on2", name="zsrc")

            # zero the pad rows (engine copies keep the fp32r-producer rule
            # happy); KTp on gpsimd, QTp on vector, both idle at start
            nc.vector.memset(ones_sc[:], 1.0)
            nc.vector.memset(zsrc[:], 0.0)
            for h in range(HPC):
                for qu in range(4):
                    hs = slice(qu * 512, (qu + 1) * 512)
                    nc.gpsimd.tensor_copy(KTp[h][64:128, hs], zsrc[64:128, :])
                    nc.vector.tensor_copy(QTp[h][64:128, hs], zsrc[64:128, :])

            nc.gpsimd.dma_start(cos_sb[:], cosT.ap()[:])
            nc.gpsimd.dma_start(sin_sb[:], sinT.ap()[:])
            if not mask_all_ones:
                nc.gpsimd.dma_start(mmul_sb[:], mmul.ap()[:])

            wq_sb = [w_pool.tile([128, HPC * HD], F32R, tag="wq", name=f"wq{_}") for _ in range(N_DTILES)]
            wk_sb = [w_pool.tile([128, HPC * HD], F32R, tag="wk", name=f"wk{_}") for _ in range(N_DTILES)]
            wv_sb = [w_pool.tile([128, HPC * HD], F32R, tag="wv", name=f"wv{_}") for _ in range(N_DTILES)]
            for d in range(N_DTILES):
                nc.sync.dma_start(wk_sb[d][:], wkT.ap()[d * 128:(d + 1) * 128, :])

            def load_xt(quarter, label):
                xt = [xt_pool.tile([128, NQ], F32R, tag="xt", name=f"xt_{label}{_}") for _ in range(N_DTILES)]
                for d in range(N_DTILES):
                    nc.sync.dma_start(
                        xt[d][:],
                        xT.ap()[d * 128:(d + 1) * 128, quarter * NQ:(quarter + 1) * NQ],
                    )
                return xt

            def rope_into(ps_slice, dsts, h0, qsl, use_act):
                """psum slice (128,512) -> RoPE -> padded head tiles rows 0:64."""
                rq = raw_pool.tile([128, NQ], F32R, tag="raw", name="rq")
                if use_act:
                    nc.scalar.copy(rq[:], ps_slice)
                else:
                    nc.vector.tensor_copy(rq[:], ps_slice)
                rot = rot_pool.tile([128, NQ], F32R, tag="rot", name="rot_t")
                for blk in range(2):
                    b0 = blk * 64
                    nc.gpsimd.dma_start(rot[b0:b0 + 32, :], rq[b0 + 32:b0 + 64, :])
                    nc.gpsimd.dma_start(rot[b0 + 32:b0 + 64, :], rq[b0:b0 + 32, :])
                nc.vector.tensor_tensor(rq[:], rq[:], cos_sb[:, qsl], OP.mult)
                nc.vector.tensor_tensor(rot[:], rot[:], sin_sb[:, qsl], OP.mult)
                nc.vector.tensor_tensor(rq[:], rq[:], rot[:], OP.add)
                nc.gpsimd.dma_start(dsts[h0][0:64, qsl], rq[0:64, :])
                nc.gpsimd.dma_start(dsts[h0 + 1][0:64, qsl], rq[64:128, :])

            def emit_q_proj(quarter, xt, use_act=False):
                qsl = slice(quarter * NQ, (quarter + 1) * NQ)
                pw = (psumS if use_act else psumW).tile(
                    [128, 1024], F32, tag="ps" if use_act else "pw", name="psq")
                for ch in range(2):
                    for d in range(N_DTILES):
                        nc.tensor.matmul(
                            pw[:, ch * 512:(ch + 1) * 512],
                            wq_sb[d][:, ch * 128:(ch + 1) * 128],
                            xt[d][:],
                            start=(d == 0), stop=(d == N_DTILES - 1),
                        )
                for ch in range(2):
                    rope_into(pw[:, ch * 512:(ch + 1) * 512], QTp, ch * 2, qsl, use_act)

            def emit_v_proj(quarter, xt, use_act=False):
                pw = (psumS if use_act else psumW).tile(
                    [128, 1024], F32, tag="ps" if use_act else "pw", name="psv")
                for tt in range(4):
                    for d in range(N_DTILES):
                        nc.tensor.matmul(
                            pw[:, tt * 256:tt * 256 + 256],
                            xt[d][:, tt * 128:(tt + 1) * 128],
                            wv_sb[d][:],
                            start=(d == 0), stop=(d == N_DTILES - 1),
                        )
                for tt in range(4):
                    g = quarter * 4 + tt
                    for h in range(HPC):
                        cp = nc.scalar.copy if use_act else (
                            lambda o, i: nc.vector.tensor_copy(o, i))
                        cp(vaug[g][:, h * 65:h * 65 + 64],
                           pw[:, tt * 256 + h * 64:tt * 256 + (h + 1) * 64])
                    nc.vector.tensor_copy(vaug[g][:, 64::65], ones_sc[:])

            # ---- A1: K projections for all quarters (K^T fully built first)
            for quarter in range(4):
                xt = load_xt(quarter, f"k{quarter}")
                if quarter == 0:
                    for d in range(N_DTILES):
                        nc.sync.dma_start(wq_sb[d][:], wqT.ap()[d * 128:(d + 1) * 128, :])
                qsl = slice(quarter * NQ, (quarter + 1) * NQ)
                ps = psumS.tile([128, 1024], F32, tag="ps", name="psk")
                for ch in range(2):
                    for d in range(N_DTILES):
                        nc.tensor.matmul(
                            ps[:, ch * 512:(ch + 1) * 512],
                            wk_sb[d][:, ch * 128:(ch + 1) * 128],
                            xt[d][:],
                            start=(d == 0), stop=(d == N_DTILES - 1),
                        )
                for ch in range(2):
                    rope_into(ps[:, ch * 512:(ch + 1) * 512], KTp, ch * 2, qsl, True)
                if quarter == 0:
                    for d in range(N_DTILES):
                        nc.sync.dma_start(wv_sb[d][:], wvT.ap()[d * 128:(d + 1) * 128, :])

            # ---- A2: Q0 + V0 (uses the same quarter-0 tokens, freshly loaded)
            xtq = load_xt(0, "q0")
            emit_q_proj(0, xtq, use_act=True)
            emit_v_proj(0, xtq, use_act=True)
            for ch in range(2):
                nc.sync.dma_start(woT_sb[ch][:], woT.ap()[ch * 128:(ch + 1) * 128, :])
            xt_next = load_xt(1, "v1")  # prefetch V quarter 1

            def emit_wo(q):
                for tt in range(4):
                    t0 = q * 512 + tt * 128
                    pw = psumW.tile([128, D], F32, tag="pw", name="pw_t")
                    for nh in range(2):
                        ns = slice(nh * 512, (nh + 1) * 512)
                        for ch in range(2):
                            nc.tensor.matmul(
                                pw[:, ns],
                                outT[ch][:, t0:t0 + 128],
                                woT_sb[ch][:, ns],
                                start=(ch == 0), stop=(ch == 1),
                            )
                    for nh in range(2):
                        yt = y_pool.tile([128, 512], F32, tag="y", name="y_t")
                        nc.vector.tensor_copy(yt[:], pw[:, nh * 512:(nh + 1) * 512])
                        nc.gpsimd.dma_start(
                            y.ap()[t0:t0 + 128, nh * 512:(nh + 1) * 512], yt[:])

            # ---- B: attention with just-in-time V / Q / Wo injection
            state = {"xt_next": xt_next}

            def emit_kgroup(q, hp, kg, pnE, pnO, qs):
                hE, hO = 2 * hp, 2 * hp + 1
                # deferred work rides the PE stream here
                if q == 0 and hp == 0 and kg in (1, 3, 5):
                    vq = (kg + 1) // 2
                    emit_v_proj(vq, state["xt_next"])
                    if vq < 3:
                        state["xt_next"] = load_xt(vq + 1, f"v{vq + 1}")
                    else:
                        state["xt_next"] = load_xt(1, "qq1")  # Q quarter 1
                if hp == int(q == 0) and kg == 2 and q < 3:
                    emit_q_proj(q + 1, state["xt_next"])
                    if q < 2:
                        state["xt_next"] = load_xt(q + 2, f"qq{q + 2}")
                if hp == 1 and kg == 0 and q > 0:
                    emit_wo(q - 1)
                psE = psumS.tile([128, 512 * KG], F32, tag="ps", name="psE")
                psO = psumS.tile([128, 512 * KG], F32, tag="ps", name="psO")
                for j in range(KG):
                    kt = kg * KG + j
                    ks = slice(kt * 128, (kt + 1) * 128)
                    js = slice(j * 512, (j + 1) * 512)
                    nc.tensor.matmul(psE[:, js], KTp[hE][:, ks], QTp[hE][:, qs],
                                     start=True, stop=True)
                    nc.tensor.matmul(psO[:, js], KTp[hO][:, ks], QTp[hO][:, qs],
                                     start=True, stop=True)
                eE = exp_pool.tile([128, 512 * KG], F32R, tag="exp", name="eE")
                eO = exp_pool.tile([128, 512 * KG], F32R, tag="exp", name="eO")
                nc.scalar.activation(eE[:], psE[:], AF.Exp, scale=SCALE)
                nc.scalar.activation(eO[:], psO[:], AF.Exp, scale=SCALE)
                if not mask_all_ones:
                    for j in range(KG):
                        kt = kg * KG + j
                        js = slice(j * 512, (j + 1) * 512)
                        nc.vector.tensor_scalar_mul(
                            eE[:, js], eE[:, js], mmul_sb[:, kt:kt + 1])
                        nc.vector.tensor_scalar_mul(
                            eO[:, js], eO[:, js], mmul_sb[:, kt:kt + 1])
                for j in range(KG):
                    kt = kg * KG + j
                    js = slice(j * 512, (j + 1) * 512)
                    nc.tensor.matmul(
                        pnE[:], vaug[kt][:, hE * 65:(hE + 1) * 65], eE[:, js],
                        start=(kt == 0), stop=(kt == N_KTILES - 1))
                    nc.tensor.matmul(
                        pnO[:], vaug[kt][:, hO * 65:(hO + 1) * 65], eO[:, js],
                        start=(kt == 0), stop=(kt == N_KTILES - 1))

            def emit_divisions(q, hp, pnE, pnO, qs):
                # stage accumulators out of PSUM first (frees pn banks),
                # then normalize via approx reciprocal + broadcast
                stgs = []
                for pn in (pnE, pnO):
                    stg = div_pool.tile([65, 512], F32, tag="stg", bufs=2, name="stg_t")
                    nc.vector.tensor_copy(stg[:], pn[:])
                    stgs.append(stg)
                recs = []
                for stg in stgs:
                    rec = div_pool.tile([1, 512], F32, tag="rec", bufs=2, name="rec_t")
                    nc.vector.reciprocal(rec[:], stg[64:65, :])
                    recs.append(rec)
                for i, (stg, rec) in enumerate(zip(stgs, recs)):
                    rbc = div_pool.tile([64, 512], F32, tag="rbc", bufs=2, name="rbc_t")
                    nc.gpsimd.partition_broadcast(rbc[:], rec[:])
                    if i == 0:
                        nc.vector.tensor_tensor(
                            outT[hp][0:64, qs], stg[0:64, :], rbc[:], OP.mult)
                    else:
                        tmp = div_pool.tile([64, 512], F32R, tag="tmp", bufs=2, name="tmp_t")
                        nc.vector.tensor_tensor(tmp[:], stg[0:64, :], rbc[:], OP.mult)
                        nc.sync.dma_start(outT[hp][64:128, qs], tmp[:])

            for q in range(N_QTILES):
                qs = slice(q * 512, (q + 1) * 512)
                for hp in range(2):
                    pnE = psumN.tile([65, 512], F32, tag="pn", name="pnE")
                    pnO = psumN.tile([65, 512], F32, tag="pn", name="pnO")
                    for kg in range(N_KTILES // KG):
                        emit_kgroup(q, hp, kg, pnE, pnO, qs)
                    emit_divisions(q, hp, pnE, pnO, qs)
            emit_wo(N_QTILES - 1)

    nc.compile()
    return nc


_CACHE = {}


def _get_program(mask_all_ones: bool):
    if mask_all_ones not in _CACHE:
        _CACHE[mask_all_ones] = _build_program(mask_all_ones)
    return _CACHE[mask_all_ones]


def _host_inputs(x, mask, Wq, Wk, Wv, Wo):
    """Build the 8 per-core input maps."""
    x = np.asarray(x, np.float32)
    mask = np.asarray(mask)
    Wq, Wk, Wv, Wo = (np.asarray(w, np.float32) for w in (Wq, Wk, Wv, Wo))

    # RoPE tables in rotate-half permuted space, repeated per 64-row block
    inv_freq = 1.0 / (10000.0 ** (np.arange(0, HD, 2, dtype=np.float32) / HD))
    ang = np.outer(np.arange(N, dtype=np.float32), inv_freq)  # (N, 32)
    cos = np.cos(ang).T.astype(np.float32)  # (32, N)
    sin = np.sin(ang).T.astype(np.float32)
    cosT = np.concatenate([cos, cos, cos, cos], 0)  # (128, N)
    sinT = np.concatenate([-sin, sin, -sin, sin], 0)

    perm = np.concatenate([np.arange(0, HD, 2), np.arange(1, HD, 2)])  # evens|odds

    xTs = [np.ascontiguousarray(x[b].T) for b in range(B)]
    in_maps = []
    for c in range(N_CORES):
        b, g = divmod(c, HPC)
        rows = []
        for h in range(HPC):
            h_abs = g * HPC + h
            rows.append(h_abs * HD + perm)
        rows = np.concatenate(rows)  # 256 permuted row indices
        vrows = np.arange(g * HPC * HD, (g + 1) * HPC * HD)  # unpermuted
        mb = mask[b].astype(np.float32).reshape(N_KTILES, 128).T.copy()  # (128,16)
        in_maps.append({
            "xT": xTs[b],
            "wqT": np.ascontiguousarray(Wq[rows].T),
            "wkT": np.ascontiguousarray(Wk[rows].T),
            "wvT": np.ascontiguousarray(Wv[vrows].T),
            "woT": np.ascontiguousarray(Wo[:, vrows].T),
            "cosT": cosT,
            "sinT": sinT,
            "mmul": np.ascontiguousarray(mb),
        })
    return in_maps


def kernel(x, mask, Wq, Wk, Wv, Wo, _want_profile=False):
    mask_all_ones = bool(np.asarray(mask).all())
    nc = _get_program(mask_all_ones)
    in_maps = _host_inputs(x, mask, Wq, Wk, Wv, Wo)
    kw = {}
    if _want_profile:
        import os
        import shutil
        shutil.rmtree("/root/problem/prof", ignore_errors=True)
        os.makedirs("/root/problem/prof", exist_ok=True)
        kw["tmpdir"] = "/root/problem/prof"
    res = run_bass_kernel_spmd(
        nc, in_maps, list(range(N_CORES)), trace=_want_profile, **kw
    )
    out = np.zeros((B, N, D), np.float32)
    for c in range(N_CORES):
        out[c // HPC] += res.results[c]["y"]
    if _want_profile:
        return out, res
    return out


# revision 27
# speedup vs baseline: 1.0203x; 1.0203x over previous
"""Multi-head attention (RoPE, 16 heads, D=1024, B=2, N=2048) on 8 trn2 cores.

Sharding: core c handles batch b = c//4 and heads [4*(c%4), 4*(c%4)+4).
Each core computes its 4 heads' attention plus its partial out-projection
(columns of Wo for its heads); host sums the 4 partials per batch.

Layout strategy (per core):
  - Q^T, K^T computed directly in (head_dim, tokens) layout; head_dim rows are
    permuted (evens then odds) via host-permuted Wq/Wk rows so RoPE becomes a
    rotate-half over partitions 0:32/32:64 of each 64-row head block.
  - scores^T tiles (k_tokens x q) via row-packed K=64 matmuls (head pairs at
    partitions 0:64 / 64:128 of shared Q^T/K^T tiles).
  - exp on ScalarE straight out of PSUM with fused 1/sqrt(hd) scale.
  - numerator matmul uses V augmented with a ones column (M=65): row 64 of the
    PSUM accumulator is the softmax denominator.
  - normalize: DVE reciprocal of row 64, gpsimd partition-broadcast, DVE mult.
  - out-projection: lhsT = normalized attention output (already transposed),
    rhs = Wo columns for this core's heads, accumulated over 2 head-pair chunks.
"""

import numpy as np

import concourse.bass as bass
import concourse.mybir as mybir
import concourse.tile as tile
from concourse import bacc
from concourse.bass_utils import run_bass_kernel_spmd

F32 = mybir.dt.float32
F32R = mybir.dt.float32r
AF = mybir.ActivationFunctionType
OP = mybir.AluOpType

B, N, D = 2, 2048, 1024
H, HD = 16, 64
HPC = 4  # heads per core
N_CORES = 8
SCALE = HD ** -0.5

N_TOKTILES = N // 128      # 16
N_KTILES = N // 128        # 16
N_QTILES = N // 512        # 4
N_DTILES = D // 128        # 8
KG = 2                     # ktiles per exp group
NEG = -1.0e30


def _build_program(mask_all_ones: bool):
    nc = bacc.Bacc("TRN2", target_bir_lowering=False, debug=False)

    xT = nc.dram_tensor("xT", [D, N], F32R, kind="ExternalInput")
    wqT = nc.dram_tensor("wqT", [D, HPC * HD], F32R, kind="ExternalInput")
    wkT = nc.dram_tensor("wkT", [D, HPC * HD], F32R, kind="ExternalInput")
    wvT = nc.dram_tensor("wvT", [D, HPC * HD], F32R, kind="ExternalInput")
    woT = nc.dram_tensor("woT", [HPC * HD, D], F32R, kind="ExternalInput")
    cosT = nc.dram_tensor("cosT", [128, N], F32, kind="ExternalInput")
    sinT = nc.dram_tensor("sinT", [128, N], F32, kind="ExternalInput")
    mmul = nc.dram_tensor("mmul", [128, N_KTILES], F32, kind="ExternalInput")
    y = nc.dram_tensor("y", [N, D], F32, kind="ExternalOutput")

    NQ = 512  # token quarter

    with tile.TileContext(nc) as tc:
        with (
            tc.tile_pool(name="qk", bufs=2 * HPC) as qk_pool,
            tc.tile_pool(name="vaug", bufs=N_TOKTILES) as v_pool,
            tc.tile_pool(name="outT", bufs=2) as outT_pool,
            tc.tile_pool(name="wo", bufs=2) as wo_pool,
            tc.tile_pool(name="mm", bufs=1) as mm_pool,
            tc.tile_pool(name="tab", bufs=2) as tab_pool,
            tc.tile_pool(name="raw", bufs=3) as raw_pool,
            tc.tile_pool(name="rot", bufs=3) as rot_pool,
            tc.tile_pool(name="xt", bufs=N_DTILES) as xt_pool,
            tc.tile_pool(name="w", bufs=N_DTILES) as w_pool,
            tc.tile_pool(name="on", bufs=1) as on_pool,
            tc.tile_pool(name="exp", bufs=3) as exp_pool,
            tc.tile_pool(name="div", bufs=4) as div_pool,
            tc.tile_pool(name="yout", bufs=2) as y_pool,
            tc.tile_pool(name="psumS", bufs=2, space="PSUM") as psumS,
            tc.tile_pool(name="psumN", bufs=2, space="PSUM") as psumN,
            tc.tile_pool(name="psumW", bufs=1, space="PSUM") as psumW,
        ):
            # QTp[h], KTp[h]: (128, N) f32r; rows 0:64 = head h, 64:128 = zeros
            # (zero-padded so every matmul has K=128 and counts as HAM-busy)
            QTp = [qk_pool.tile([128, N], F32R, tag="qk", name=f"QTp{_}") for _ in range(HPC)]
            KTp = [qk_pool.tile([128, N], F32R, tag="qk", name=f"KTp{_}") for _ in range(HPC)]
            vaug = [
                v_pool.tile([128, HPC * (HD + 1)], F32R, tag="vaug", name=f"vaug{_}")
                for _ in range(N_TOKTILES)
            ]
            outT = [outT_pool.tile([128, N], F32R, tag="outT", name=f"outT{_}") for _ in range(2)]
            woT_sb = [wo_pool.tile([128, D], F32R, tag="wo", name=f"woTsb{_}") for _ in range(2)]
            mmul_sb = mm_pool.tile([128, N_KTILES], F32)
            cos_sb = tab_pool.tile([128, N], F32, tag="tab")
            sin_sb = tab_pool.tile([128, N], F32, tag="tab")
            ones_sc = on_pool.tile([128, HPC], F32, tag="on1", name="ones_sc")
            zsrc = on_pool.tile([128, 512], F32, tag="on2", name="zsrc")

            # zero the pad rows (engine copies keep the fp32r-producer rule
            # happy); KTp on gpsimd, QTp on vector, both idle at start
            nc.vector.memset(ones_sc[:], 1.0)
            nc.vector.memset(zsrc[:], 0.0)
            for h in range(HPC):
                for qu in range(4):
                    hs = slice(qu * 512, (qu + 1) * 512)
                    nc.gpsimd.tensor_copy(KTp[h][64:128, hs], zsrc[64:128, :])
                    nc.vector.tensor_copy(QTp[h][64:128, hs], zsrc[64:128, :])

            nc.gpsimd.dma_start(cos_sb[:], cosT.ap()[:])
            nc.gpsimd.dma_start(sin_sb[:], sinT.ap()[:])
            if not mask_all_ones:
                nc.gpsimd.dma_start(mmul_sb[:], mmul.ap()[:])

            wq_sb = [w_pool.tile([128, HPC * HD], F32R, tag="wq", name=f"wq{_}") for _ in range(N_DTILES)]
            wk_sb = [w_pool.tile([128, HPC * HD], F32R, tag="wk", name=f"wk{_}") for _ in range(N_DTILES)]
            wv_sb = [w_pool.tile([128, HPC * HD], F32R, tag="wv", name=f"wv{_}") for _ in range(N_DTILES)]
            for d in range(N_DTILES):
                nc.sync.dma_start(wk_sb[d][:], wkT.ap()[d * 128:(d + 1) * 128, :])

            def load_xt(quarter, label):
                xt = [xt_pool.tile([128, NQ], F32R, tag="xt", name=f"xt_{label}{_}") for _ in range(N_DTILES)]
                for d in range(N_DTILES):
                    nc.sync.dma_start(
                        xt[d][:],
                        xT.ap()[d * 128:(d + 1) * 128, quarter * NQ:(quarter + 1) * NQ],
                    )
                return xt

            def rope_into(ps_slice, dsts, h0, qsl, use_act):
                """psum slice (128,512) -> RoPE -> padded head tiles rows 0:64."""
                rq = raw_pool.tile([128, NQ], F32R, tag="raw", name="rq")
                if use_act:
                    nc.scalar.copy(rq[:], ps_slice)
                else:
                    nc.vector.tensor_copy(rq[:], ps_slice)
                rot = rot_pool.tile([128, NQ], F32R, tag="rot", name="rot_t")
                for blk in range(2):
                    b0 = blk * 64
                    nc.gpsimd.dma_start(rot[b0:b0 + 32, :], rq[b0 + 32:b0 + 64, :])
                    nc.gpsimd.dma_start(rot[b0 + 32:b0 + 64, :], rq[b0:b0 + 32, :])
                nc.vector.tensor_tensor(rq[:], rq[:], cos_sb[:, qsl], OP.mult)
                nc.vector.tensor_tensor(rot[:], rot[:], sin_sb[:, qsl], OP.mult)
                nc.vector.tensor_tensor(rq[:], rq[:], rot[:], OP.add)
                nc.gpsimd.dma_start(dsts[h0][0:64, qsl], rq[0:64, :])
                nc.gpsimd.dma_start(dsts[h0 + 1][0:64, qsl], rq[64:128, :])

            def emit_q_proj(quarter, xt, use_act=False):
                qsl = slice(quarter * NQ, (quarter + 1) * NQ)
                pw = (psumS if use_act else psumW).tile(
                    [128, 1024], F32, tag="ps" if use_act else "pw", name="psq")
                for ch in range(2):
                    for d in range(N_DTILES):
                        nc.tensor.matmul(
                            pw[:, ch * 512:(ch + 1) * 512],
                            wq_sb[d][:, ch * 128:(ch + 1) * 128],
                            xt[d][:],
                            start=(d == 0), stop=(d == N_DTILES - 1),
                        )
                for ch in range(2):
                    rope_into(pw[:, ch * 512:(ch + 1) * 512], QTp, ch * 2, qsl, use_act)

            def emit_v_proj(quarter, xt, use_act=False):
                pw = (psumS if use_act else psumW).tile(
                    [128, 1024], F32, tag="ps" if use_act else "pw", name="psv")
                for tt in range(4):
                    for d in range(N_DTILES):
                        nc.tensor.matmul(
                            pw[:, tt * 256:tt * 256 + 256],
                            xt[d][:, tt * 128:(tt + 1) * 128],
                            wv_sb[d][:],
                            start=(d == 0), stop=(d == N_DTILES - 1),
                        )
                for tt in range(4):
                    g = quarter * 4 + tt
                    for h in range(HPC):
                        cp = nc.scalar.copy if use_act else (
                            lambda o, i: nc.vector.tensor_copy(o, i))
                        cp(vaug[g][:, h * 65:h * 65 + 64],
                           pw[:, tt * 256 + h * 64:tt * 256 + (h + 1) * 64])
                    nc.vector.tensor_copy(vaug[g][:, 64::65], ones_sc[:])

            # ---- A1: K projections for all quarters (K^T fully built first)
            for quarter in range(4):
                xt = load_xt(quarter, f"k{quarter}")
                if quarter == 0:
                    for d in range(N_DTILES):
                        nc.sync.dma_start(wq_sb[d][:], wqT.ap()[d * 128:(d + 1) * 128, :])
                qsl = slice(quarter * NQ, (quarter + 1) * NQ)
                ps = psumS.tile([128, 1024], F32, tag="ps", name="psk")
                for ch in range(2):
                    for d in range(N_DTILES):
                        nc.tensor.matmul(
                            ps[:, ch * 512:(ch + 1) * 512],
                            wk_sb[d][:, ch * 128:(ch + 1) * 128],
                            xt[d][:],
                            start=(d == 0), stop=(d == N_DTILES - 1),
                        )
                for ch in range(2):
                    rope_into(ps[:, ch * 512:(ch + 1) * 512], KTp, ch * 2, qsl, True)
                if quarter == 0:
                    for d in range(N_DTILES):
                        nc.sync.dma_start(wv_sb[d][:], wvT.ap()[d * 128:(d + 1) * 128, :])

            # ---- A2: Q0 + V0 (uses the same quarter-0 tokens, freshly loaded)
            xtq = load_xt(0, "q0")
            emit_q_proj(0, xtq, use_act=True)
            emit_v_proj(0, xtq, use_act=True)
            for ch in range(2):
                nc.sync.dma_start(woT_sb[ch][:], woT.ap()[ch * 128:(ch + 1) * 128, :])
            xt_next = load_xt(1, "v1")  # prefetch V quarter 1

            def emit_wo(q):
                for tt in range(4):
                    t0 = q * 512 + tt * 128
                    pw = psumW.tile([128, D], F32, tag="pw", name="pw_t")
                    for nh in range(2):
                        ns = slice(nh * 512, (nh + 1) * 512)
                        for ch in range(2):
                            nc.tensor.matmul(
                                pw[:, ns],
                                outT[ch][:, t0:t0 + 128],
                                woT_sb[ch][:, ns],
                                start=(ch == 0), stop=(ch == 1),
                            )
                    for nh in range(2):
                        yt = y_pool.tile([128, 512], F32, tag="y", name="y_t")
                        nc.vector.tensor_copy(yt[:], pw[:, nh * 512:(nh + 1) * 512])
                        nc.gpsimd.dma_start(
                            y.ap()[t0:t0 + 128, nh * 512:(nh + 1) * 512], yt[:])

            # ---- B: attention with just-in-time V / Q / Wo injection
            state = {"xt_next": xt_next}

            def emit_kgroup(q, hp, kg, pnE, pnO, qs):
                hE, hO = 2 * hp, 2 * hp + 1
                # deferred work rides the PE stream here
                if q == 0 and hp == 0 and kg in (1, 3, 5):
                    vq = (kg + 1) // 2
                    emit_v_proj(vq, state["xt_next"])
                    if vq < 3:
                        state["xt_next"] = load_xt(vq + 1, f"v{vq + 1}")
                    else:
                        state["xt_next"] = load_xt(1, "qq1")  # Q quarter 1
                if hp == int(q == 0) and kg == 2 and q < 3:
                    emit_q_proj(q + 1, state["xt_next"])
                    if q < 2:
                        state["xt_next"] = load_xt(q + 2, f"qq{q + 2}")
                if hp == 1 and kg == 0 and q > 0:
                    emit_wo(q - 1)
                psE = psumS.tile([128, 512 * KG], F32, tag="ps", name="psE")
                psO = psumS.tile([128, 512 * KG], F32, tag="ps", name="psO")
                for j in range(KG):
                    kt = kg * KG + j
                    ks = slice(kt * 128, (kt + 1) * 128)
                    js = slice(j * 512, (j + 1) * 512)
                    nc.tensor.matmul(psE[:, js], KTp[hE][:, ks], QTp[hE][:, qs],
                                     start=True, stop=True)
                    nc.tensor.matmul(psO[:, js], KTp[hO][:, ks], QTp[hO][:, qs],
                                     start=True, stop=True)
                eE = exp_pool.tile([128, 512 * KG], F32R, tag="exp", name="eE")
                eO = exp_pool.tile([128, 512 * KG], F32R, tag="exp", name="eO")
                nc.scalar.activation(eE[:], psE[:], AF.Exp, scale=SCALE)
                nc.scalar.activation(eO[:], psO[:], AF.Exp, scale=SCALE)
                if not mask_all_ones:
                    for j in range(KG):
                        kt = kg * KG + j
                        js = slice(j * 512, (j + 1) * 512)
                        nc.vector.tensor_scalar_mul(
                            eE[:, js], eE[:, js], mmul_sb[:, kt:kt + 1])
                        nc.vector.tensor_scalar_mul(
                            eO[:, js], eO[:, js], mmul_sb[:, kt:kt + 1])
                for j in range(KG):
                    kt = kg * KG + j
                    js = slice(j * 512, (j + 1) * 512)
                    nc.tensor.matmul(
                        pnE[:], vaug[kt][:, hE * 65:(hE + 1) * 65], eE[:, js],
                        start=(kt == 0), stop=(kt == N_KTILES - 1))
                    nc.tensor.matmul(
                        pnO[:], vaug[kt][:, hO * 65:(hO + 1) * 65], eO[:, js],
                        start=(kt == 0), stop=(kt == N_KTILES - 1))

            def emit_divisions(q, hp, pnE, pnO, qs):
                # stage accumulators out of PSUM first (frees pn banks),
                # then normalize via approx reciprocal + broadcast
                stgs = []
                for pn in (pnE, pnO):
                    stg = div_pool.tile([65, 512], F32, tag="stg", bufs=2, name="stg_t")
                    nc.vector.tensor_copy(stg[:], pn[:])
                    stgs.append(stg)
                recs = []
                for stg in stgs:
                    # 1/d = exp(-ln d) on ScalarE: cheap, high precision, and
                    # keeps the reciprocal off the (busy) vector queue
                    rec = div_pool.tile([1, 512], F32, tag="rec", bufs=2, name="rec_t")
                    nc.scalar.activation(rec[:], stg[64:65, :], AF.Ln)
                    nc.scalar.activation(rec[:], rec[:], AF.Exp, scale=-1.0)
                    recs.append(rec)
                for i, (stg, rec) in enumerate(zip(stgs, recs)):
                    rbc = div_pool.tile([64, 512], F32, tag="rbc", bufs=2, name="rbc_t")
                    nc.gpsimd.partition_broadcast(rbc[:], rec[:])
                    if i == 0:
                        nc.vector.tensor_tensor(
                            outT[hp][0:64, qs], stg[0:64, :], rbc[:], OP.mult)
                    else:
                        tmp = div_pool.tile([64, 512], F32R, tag="tmp", bufs=2, name="tmp_t")
                        nc.vector.tensor_tensor(tmp[:], stg[0:64, :], rbc[:], OP.mult)
                        nc.sync.dma_start(outT[hp][64:128, qs], tmp[:])

            for q in range(N_QTILES):
                qs = slice(q * 512, (q + 1) * 512)
                for hp in range(2):
                    pnE = psumN.tile([65, 512], F32, tag="pn", name="pnE")
                    pnO = psumN.tile([65, 512], F32, tag="pn", name="pnO")
                    for kg in range(N_KTILES // KG):
                        emit_kgroup(q, hp, kg, pnE, pnO, qs)
                    emit_divisions(q, hp, pnE, pnO, qs)
            emit_wo(N_QTILES - 1)

    nc.compile()
    return nc


_CACHE = {}


def _get_program(mask_all_ones: bool):
    if mask_all_ones not in _CACHE:
        _CACHE[mask_all_ones] = _build_program(mask_all_ones)
    return _CACHE[mask_all_ones]


def _host_inputs(x, mask, Wq, Wk, Wv, Wo):
    """Build the 8 per-core input maps."""
    x = np.asarray(x, np.float32)
    mask = np.asarray(mask)
    Wq, Wk, Wv, Wo = (np.asarray(w, np.float32) for w in (Wq, Wk, Wv, Wo))

    # RoPE tables in rotate-half permuted space, repeated per 64-row block
    inv_freq = 1.0 / (10000.0 ** (np.arange(0, HD, 2, dtype=np.float32) / HD))
    ang = np.outer(np.arange(N, dtype=np.float32), inv_freq)  # (N, 32)
    cos = np.cos(ang).T.astype(np.float32)  # (32, N)
    sin = np.sin(ang).T.astype(np.float32)
    cosT = np.concatenate([cos, cos, cos, cos], 0)  # (128, N)
    sinT = np.concatenate([-sin, sin, -sin, sin], 0)

    perm = np.concatenate([np.arange(0, HD, 2), np.arange(1, HD, 2)])  # evens|odds

    xTs = [np.ascontiguousarray(x[b].T) for b in range(B)]
    in_maps = []
    for c in range(N_CORES):
        b, g = divmod(c, HPC)
        rows = []
        for h in range(HPC):
            h_abs = g * HPC + h
            rows.append(h_abs * HD + perm)
        rows = np.concatenate(rows)  # 256 permuted row indices
        vrows = np.arange(g * HPC * HD, (g + 1) * HPC * HD)  # unpermuted
        mb = mask[b].astype(np.float32).reshape(N_KTILES, 128).T.copy()  # (128,16)
        in_maps.append({
            "xT": xTs[b],
            "wqT": np.ascontiguousarray(Wq[rows].T),
            "wkT": np.ascontiguousarray(Wk[rows].T),
            "wvT": np.ascontiguousarray(Wv[vrows].T),
            "woT": np.ascontiguousarray(Wo[:, vrows].T),
            "cosT": cosT,
            "sinT": sinT,
            "mmul": np.ascontiguousarray(mb),
        })
    return in_maps


def kernel(x, mask, Wq, Wk, Wv, Wo, _want_profile=False):
    mask_all_ones = bool(np.asarray(mask).all())
    nc = _get_program(mask_all_ones)
    in_maps = _host_inputs(x, mask, Wq, Wk, Wv, Wo)
    kw = {}
    if _want_profile:
        import os
        import shutil
        shutil.rmtree("/root/problem/prof", ignore_errors=True)
        os.makedirs("/root/problem/prof", exist_ok=True)
        kw["tmpdir"] = "/root/problem/prof"
    res = run_bass_kernel_spmd(
        nc, in_maps, list(range(N_CORES)), trace=_want_profile, **kw
    )
    out = np.zeros((B, N, D), np.float32)
    for c in range(N_CORES):
        out[c // HPC] += res.results[c]["y"]
    if _want_profile:
        return out, res
    return out
